# revision 52
# baseline (speedup 1.0000x reference)
"""EnergyBasedVAD Trainium2 kernel.

Input:  waveform (32, 960000) f32.
Output: (32, 3749) bool VAD mask.

Sharding: pure data parallel — 4 batch rows per core across 8 cores.

Device computes short-time energy (the memory-bound part: 123 MB of
waveform reads). Each row of 960000 samples is 125 partitions x 7680
samples (30 blocks of 256, no halo), loaded in tapered column slices
(ascending sizes on the first row for an early pipeline start,
descending on the last row for a tiny post-DMA tail). Every input
transfer carries 128 partition-lines (the HWDGE splits a transfer's
lines over count/(largest divisor <= 16) SDMA engines: 128 lines ->
all 16, 125 lines -> only 5; partitions 125-127 read pad garbage).
Slices alternate between the two HWDGE rings (sync/scalar): two
descriptor generators in parallel sustain ~410 GB/s where one ring is
DGE-bound at ~340. Row r+1's dma_starts are emitted before row r's
compute so the scalar-ring issues don't queue behind ACTIVATEs, and
the leading waitless DMAs are hoisted into the prologue block ahead
of the framework's const-memset barrier. Outputs ride SWDGE (gpsimd)
so they never head-of-line-block an input ring.

Each slice is squared on ACT into bf16 (mean's 1/512 folded into the
activation scale), then block-summed on DVE with two pairwise bf16
tensor_tensor adds (2x mode; tensor_reduce is capped at 1x) and a
f=16 reduce into a per-row [125, 30] f32 tile of 256-sample block
energies, DMA'd out raw. The final pair-add (frame t = block t +
block t+1, which crosses partition boundaries every 30th frame) runs
on host over the (32, 3750) block sums.

Host also computes the 20%-quantile threshold and the hysteresis
segment state machine on the (32, 3749) energies — 0.01% of the bytes.
"""

import math
import numpy as np

import concourse.bass as bass
import concourse.bacc as bacc
import concourse.mybir as mybir
from concourse.bass_utils import run_bass_kernel_spmd
from concourse.tile import TileContext

N_CORES = 8
B, S = 32, 960000
ROWS = B // N_CORES          # 4 rows per core
PV = 125                     # partitions per row
SEG = 7680                   # samples per partition = 30 blocks of 256
NBLK_P = 30                  # 256-blocks per partition
NBLK = S // 256              # 3750 block sums per row
T = (S - 512) // 256 + 1     # 3749 output frames
# DMA slices carry 128 partition-lines (the HWDGE splits a transfer's lines
# across SDMA engines as count/(largest divisor <= 16): 128 lines -> 16
# engines, 125 lines -> only 5). Partitions 125-127 read past the row into
# the pad (garbage, discarded); pad covers the last row's overhang.
PDMA = 128
FLAT = ROWS * S + (PDMA - PV) * SEG

# per-row column-slice plans (blocks of 256 samples per slice). Rows 0-2 use
# three equal slices; row 3 tapers so the after-last-DMA compute tail is tiny.
SLICE_PLAN = [
    [10, 10, 10],         # flat: fewer transfers beats an ascending taper —
    [10, 10, 10],         # HWDGE descriptor-gen costs ~3.6us per 128-line
    [10, 10, 10],         # transfer regardless of size, so a finer ramp
    [10, 8, 6, 4, 2],     # starves the stream; descending tail stays (tiny
]                         # after-last-DMA compute chain)
MAX_CSL = max(max(p) for p in SLICE_PLAN) * 256

SILENCE_FRAMES = 18
MIN_SPEECH_FRAMES = 6
ENERGY_THRESHOLD = 0.01

_CACHE = {}


def _build(repeat: int = 1, slice_plan=None, rings=("sync", "scalar"), wav_bufs=8, sq_bufs=4, stagger=True, bf16_tree=True, gp_square=False, dve_square_last=0, last_row_sync=2):
    slice_plan = slice_plan or SLICE_PLAN
    max_csl = max(max(p) for p in slice_plan) * 256
    # no partition_id: the program is identical on every core (pure data
    # parallel) — skipping it drops the per-engine TENSOR_LOAD preamble
    nc = bacc.Bacc(None, enable_partition_id=False)
    wav = nc.declare_dram_parameter("waveform", [FLAT], mybir.dt.float32, isOutput=False)
    eout = nc.declare_dram_parameter("energy", [ROWS, NBLK], mybir.dt.float32, isOutput=True)

    inv = 1.0 / math.sqrt(512.0)
    sq_t = mybir.ActivationFunctionType.Square

    with TileContext(nc) as tc:
        with (
            tc.tile_pool(name="wav", bufs=wav_bufs) as wav_pool,
            tc.tile_pool(name="sq", bufs=max(sq_bufs, max(len(p) for p in slice_plan))) as sq_pool,
            tc.tile_pool(name="c64", bufs=2) as c64_pool,
            tc.tile_pool(name="c256", bufs=2) as c256_pool,
            tc.tile_pool(name="h32", bufs=3) as h32_pool,
            tc.tile_pool(name="h16", bufs=3) as h16_pool,
        ):
            ns = 0

            def emit_dmas(r):
                nonlocal ns
                tiles = []
                blk0 = 0
                for nblk in slice_plan[r]:
                    csl = nblk * 256
                    wt = wav_pool.tile([128, max_csl], mybir.dt.float32)
                    # alternate descriptor-generator rings: parallel DGEs keep
                    # the 16 SDMA engines fed (~413 GB/s vs ~340 single-ring,
                    # which is DGE-bound)
                    if last_row_sync and r >= ROWS - last_row_sync:
                        eng = nc.sync   # keep the tail issues off the busy ACT
                    else:
                        eng = getattr(nc, rings[ns % len(rings)])
                    ns += 1
                    eng.dma_start(
                        out=wt[:, 0:csl],
                        in_=bass.AP(wav, r * S + blk0 * 256, [[SEG, PDMA], [1, csl]]),
                    )
                    tiles.append((wt, blk0, nblk))
                    blk0 += nblk
                return tiles

            def emit_compute(r, tiles):
                c64t = c64_pool.tile([128, NBLK_P * 4], mybir.dt.float32)
                for si, (wt, blk0, nblk) in enumerate(tiles):
                    csl = nblk * 256
                    if bf16_tree:
                        # ACT squares to bf16; two pairwise bf16 tensor_tensor
                        # adds run in DVE 2x mode (tensor_reduce is capped at
                        # 1x), then a f=16 reduce: ~1.85us vs 2.82us per
                        # 10-block slice on DVE
                        sq = sq_pool.tile([128, max_csl], mybir.dt.bfloat16)
                        if gp_square and r == ROWS - 2 and si == len(tiles) - 1:
                            # offload one end-game square to the idle GPSIMD:
                            # relieves ACT's tail backlog by one ACTIVATE.
                            # Unscaled x*x — the 1/512 mean is applied on host.
                            nc.gpsimd.tensor_tensor(
                                sq[0:PV, 0:csl], wt[0:PV, 0:csl], wt[0:PV, 0:csl],
                                op=mybir.AluOpType.mult,
                            )
                        elif dve_square_last and r == ROWS - 1 and si >= len(tiles) - dve_square_last:
                            # square the final slices on DVE itself: the end of
                            # the dependency chain no longer waits on the
                            # backlogged ACT sequencer at all
                            nc.vector.tensor_tensor(
                                sq[0:PV, 0:csl], wt[0:PV, 0:csl], wt[0:PV, 0:csl],
                                op=mybir.AluOpType.mult,
                            )
                        else:
                            nc.scalar.activation(sq[0:PV, 0:csl], wt[0:PV, 0:csl], sq_t)
                        n = csl // 64
                        sqv = sq[0:PV, 0:csl].rearrange("p (n f) -> p n f", f=64)
                        h32 = h32_pool.tile([128, max_csl // 2], mybir.dt.bfloat16)
                        h32v = h32[0:PV, 0:csl // 2].rearrange("p (n f) -> p n f", f=32)
                        nc.vector.tensor_add(h32v, sqv[:, :, 0:32], sqv[:, :, 32:64])
                        h16 = h16_pool.tile([128, max_csl // 4], mybir.dt.bfloat16)
                        h16v = h16[0:PV, 0:csl // 4].rearrange("p (n f) -> p n f", f=16)
                        nc.vector.tensor_add(h16v, h32v[:, :, 0:16], h32v[:, :, 16:32])
                        nc.vector.reduce_sum(
                            c64t[0:PV, 4 * blk0: 4 * (blk0 + nblk)], h16v,
                            axis=mybir.AxisListType.X,
                        )
                    else:
                        sq = sq_pool.tile([128, max_csl], mybir.dt.float32)
                        nc.scalar.activation(sq[0:PV, 0:csl], wt[0:PV, 0:csl], sq_t)
                        nc.vector.reduce_sum(
                            c64t[0:PV, 4 * blk0: 4 * (blk0 + nblk)],
                            sq[0:PV, 0:csl].rearrange("p (n f) -> p n f", f=64),
                            axis=mybir.AxisListType.X,
                        )
                c256 = c256_pool.tile([128, NBLK_P], mybir.dt.float32)
                # split the final row's block-sum reduce + output so the last
                # chunk (2 blocks) is tiny: teardown waits on a 1KB transfer
                splits = [(0, NBLK_P)] if r != ROWS - 1 else [(0, 28), (28, NBLK_P)]
                eo = eout[r].rearrange("(p x) -> p x", p=PV)
                for b0, b1 in splits:
                    nc.vector.reduce_sum(
                        c256[0:PV, b0:b1],
                        c64t[0:PV, 4 * b0: 4 * b1].rearrange("p (n f) -> p n f", f=4),
                        axis=mybir.AxisListType.X,
                    )
                    # outputs ride SWDGE (gpsimd) so they never head-of-line-
                    # block an input ring mid-stream; the final row's go out on
                    # the sync HWDGE ring (idle by then, lower latency)
                    if r == ROWS - 1:
                        # final row: idle sync HWDGE ring, packed into a single
                        # packet to minimize completion latency (tiny transfer)
                        nc.sync.dma_start(
                            out=eo[:, b0:b1], in_=c256[0:PV, b0:b1],
                            single_packet=True,
                        )
                    else:
                        nc.gpsimd.dma_start(out=eo[:, b0:b1], in_=c256[0:PV, b0:b1])

            if stagger:
                # issue row r+1's DMAs (program order) before row r's compute
                # so scalar-ring dma_starts don't queue behind ACTIVATEs
                pending = None
                for i in range(ROWS * repeat):
                    r = i % ROWS
                    tiles = emit_dmas(r)
                    if pending is not None:
                        emit_compute(*pending)
                    pending = (r, tiles)
                emit_compute(*pending)
            else:
                for i in range(ROWS * repeat):
                    r = i % ROWS
                    emit_compute(r, emit_dmas(r))

    # Hoist the leading run of waitless input DMAs (and the ACT table load)
    # from the body block into the prologue block, ahead of the const-memset
    # barrier: the first transfers then issue ~2us earlier, overlapping the
    # framework preamble. Safe: their target tiles are fresh (no prior
    # readers/writers) and DMA-completion semaphores are load-time zeroed.
    f = nc.m.functions[0]
    if len(f.blocks) >= 2:
        blk0, blk1 = f.blocks[0], f.blocks[1]
        lead = []
        for ins in list(blk1.instructions):
            if isinstance(ins, (mybir.InstDMACopy, mybir.InstLoadActFuncSet)):
                lead.append(ins)
            else:
                break
        if lead and isinstance(blk0.instructions[0], mybir.InstCall):
            for ins in lead:
                blk1.instructions.remove(ins)
            # before the InstCall preamble: the first transfers issue at ~t=0,
            # overlapping the NRT rendezvous/TENSOR_LOAD boilerplate entirely
            blk0.instructions[0:0] = lead

    nc.finalize()
    return nc


def _in_maps(waveform: np.ndarray):
    w = np.ascontiguousarray(waveform, dtype=np.float32)
    pad = np.zeros(FLAT - ROWS * S, np.float32)
    return [
        {"waveform": np.concatenate([w[c * ROWS:(c + 1) * ROWS].ravel(), pad])}
        for c in range(N_CORES)
    ]


def _run_device(waveform: np.ndarray, trace: bool = False, trace_cores=None):
    if "nc" not in _CACHE:
        _CACHE["nc"] = _build()
    nc = _CACHE["nc"]
    res = run_bass_kernel_spmd(
        nc, _in_maps(waveform), core_ids=list(range(N_CORES)), trace=trace,
        trace_cores=trace_cores,
    )
    blocks = np.concatenate([res.results[c]["energy"] for c in range(N_CORES)], axis=0)
    # frame t = (block t + block t+1) / 512: device outputs raw block sums of
    # squares; the frame-mean scale is applied here
    energy = (blocks[:, :-1] + blocks[:, 1:]) * np.float32(1.0 / 512.0)
    return energy, res


def _vad_from_energy(e: np.ndarray) -> np.ndarray:
    """Threshold + hysteresis state machine, faithful to the reference."""
    n = e.shape[1]
    out = np.zeros((e.shape[0], n), dtype=bool)
    for b in range(e.shape[0]):
        s = np.sort(e[b])
        nzero = int((s <= 0).sum())
        nz = n - nzero
        if nz > 0:
            pos = np.float32(0.2) * np.float32(nz - 1)
            lo = int(np.floor(pos))
            hi = int(np.ceil(pos))
            frac = np.float32(pos) - np.float32(lo)
            ilo = min(max(nzero + lo, 0), n - 1)
            ihi = min(max(nzero + hi, 0), n - 1)
            thr = np.float32(s[ilo] * (np.float32(1.0) - frac) + s[ihi] * frac)
        else:
            thr = np.float32(ENERGY_THRESHOLD)
        m = e[b] > thr
        t = np.nonzero(m)[0]
        if len(t) == 0:
            continue
        grp = np.concatenate([[0], (np.diff(t) > SILENCE_FRAMES).cumsum()])
        for g in range(grp[-1] + 1):
            tg = t[grp == g]
            first, last = int(tg[0]), int(tg[-1])
            if last >= n - SILENCE_FRAMES:
                st, en = first, n      # trailing open segment
            else:
                st, en = first, last   # closed: end excludes last speech frame
            if en - st >= MIN_SPEECH_FRAMES:
                out[b, st:en] = True
    return out


def kernel(waveform: np.ndarray, _trace: bool = False) -> np.ndarray:
    energy, res = _run_device(waveform, trace=_trace)
    _CACHE["last_result"] = res
    return _vad_from_energy(energy)
